# revision 17
# baseline (speedup 1.0000x reference)
"""CG sparse product (CGNet) Trainium2 kernel.

Math: for each (l, l1, l2) satisfying |l1-l2| <= l <= l1+l2 (LMAX=5),
complex Kronecker product of per-l fragments followed by Clebsch-Gordan
contraction.  Data-parallel over batch: 32 examples -> 4 per core on 8 cores.

Per-core pipeline:
  1. Strided DMAs load fs into Astack [36=(l,m), (b=4, c=2, t=32)] SBUF tile.
  2. Replication matmuls (0/1 constant weights) expand Astack rows into
     kron-row-aligned S/B tiles [Kg, 256] per group (unordered (l1<=l2) pairs
     packed into groups with sum K1*K2 <= 128, sum M <= 128).
  3. DVE/GPSIMD tensor_tensor broadcast ops form the 4 real Kronecker
     products P1=ar*br, P2=ai*bi, P3=ar*bi, P4=ai*br and kr=P1-P2, ki=P3+P4
     as [Kg, 1024=(t1,t2)] tiles.
  4. TensorE matmuls apply stacked CG matrices: psum[Mg, 1024] per comp.
     Reversed orderings (l2,l1) reuse the same kron tiles via row-permuted
     constants + (t1<->t2)-swapped access patterns.
  5. PE transposes (8 phases, u = 8*p + r) -> psum_t [128, (r=8, Mg)].
  6. Strided copies assemble final-layout staging tiles ST[b][l]
     [128, (pair, r, m, c)]; one DMA per (b, l) writes contiguous DRAM.
"""

import sys

sys.path.insert(0, "/opt/trn_rl_repo")

from contextlib import ExitStack
from math import factorial as _ft

import numpy as np

import concourse.bass as bass
import concourse.tile as tile
from concourse import bacc, mybir
from concourse.bass_utils import run_bass_kernel_spmd

dt = mybir.dt

LMAX = 5
TAU = 32
BATCH = 32
N_CORES = 8
B_LOC = BATCH // N_CORES  # 4
NL = LMAX + 1

LTUPLES = [
    (l, l1, l2)
    for l in range(NL)
    for l1 in range(NL)
    for l2 in range(NL)
    if abs(l1 - l2) <= l <= l1 + l2
]

# ---------------------------------------------------------------- CG numerics


def _cg_coef(j1, m1, j2, m2, j3, m3):
    if m1 + m2 != m3 or not (abs(j1 - j2) <= j3 <= j1 + j2):
        return 0.0
    pref = np.sqrt(
        (2 * j3 + 1)
        * _ft(j3 + j1 - j2)
        * _ft(j3 - j1 + j2)
        * _ft(j1 + j2 - j3)
        / _ft(j1 + j2 + j3 + 1)
    )
    pref *= np.sqrt(
        _ft(j3 + m3)
        * _ft(j3 - m3)
        * _ft(j1 - m1)
        * _ft(j1 + m1)
        * _ft(j2 - m2)
        * _ft(j2 + m2)
    )
    s = 0.0
    for k in range(j1 + j2 - j3 + 1):
        args = (
            k,
            j1 + j2 - j3 - k,
            j1 - m1 - k,
            j2 + m2 - k,
            j3 - j2 + m1 + k,
            j3 - j1 - m2 + k,
        )
        if min(args) < 0:
            continue
        den = 1.0
        for a in args:
            den *= _ft(a)
        s += (-1.0) ** k / den
    return pref * s


def _cg_mat(l1, l2, l):
    M = np.zeros((2 * l + 1, (2 * l1 + 1) * (2 * l2 + 1)), dtype=np.float64)
    for m in range(-l, l + 1):
        for m1 in range(-l1, l1 + 1):
            m2 = m - m1
            if abs(m2) <= l2:
                M[m + l, (m1 + l1) * (2 * l2 + 1) + (m2 + l2)] = _cg_coef(
                    l1, m1, l2, m2, l, m
                )
    return M.astype(np.float32)


CG_MATS = {(l1, l2, l): _cg_mat(l1, l2, l) for (l, l1, l2) in LTUPLES}

# ------------------------------------------------------------- layout helpers

K_L = [2 * l + 1 for l in range(NL)]
OFF_L = []  # fs column offset (elems) of each l fragment
_off = 0
for l in range(NL):
    OFF_L.append(_off)
    _off += TAU * K_L[l] * 2
TOTAL_IN = _off  # 2304

AROW = []  # Astack row for (l, m)
_r = 0
AROW_OF = {}
for l in range(NL):
    for m in range(K_L[l]):
        AROW_OF[(l, m)] = _r
        _r += 1
N_AROWS = _r  # 36

# tuple ordering / output offsets
TUP_MBASE = {}  # (l,l1,l2) -> sum of (2l'+1) over preceding tuples
_mb = 0
for t in LTUPLES:
    TUP_MBASE[t] = _mb
    _mb += 2 * t[0] + 1
M_TOTAL = _mb  # 771
OUT_N = 1024 * M_TOTAL  # per-batch rows of [.,2] output

# per-l pair lists in LTUPLES order and DRAM offsets
PAIRS_OF_L = {l: [(l1, l2) for (ll, l1, l2) in LTUPLES if ll == l] for l in range(NL)}
LBLOCK_BASE = {}  # l -> flat f32-elem offset of l block within one batch row
_fb = 0
for l in range(NL):
    LBLOCK_BASE[l] = _fb
    _fb += len(PAIRS_OF_L[l]) * 1024 * K_L[l] * 2
assert _fb == OUT_N * 2

# ------------------------------------------------------- group packing (host)

UNORD = [(la, lb) for la in range(NL) for lb in range(la, NL)]


def _valid_l(l1, l2):
    return [l for l in range(NL) if abs(l1 - l2) <= l <= l1 + l2]


def _pair_m(la, lb):
    m = sum(2 * l + 1 for l in _valid_l(la, lb))
    if la != lb:
        m *= 2
    return m


def _nat_m(la, lb):
    return sum(2 * l + 1 for l in _valid_l(la, lb))


def _rev_m(la, lb):
    return 0 if la == lb else _nat_m(la, lb)


def _ceil32(x):
    return (x + 31) // 32 * 32


def _pack_groups():
    items = sorted(UNORD, key=lambda p: -(2 * p[0] + 1) * (2 * p[1] + 1))
    groups = []  # list of dicts {pairs, K, natM, revM}
    for la, lb in items:
        k = (2 * la + 1) * (2 * lb + 1)
        nm, rm = _nat_m(la, lb), _rev_m(la, lb)
        placed = False
        for g in groups:
            if (
                g["K"] + k <= 128
                and _ceil32(g["natM"] + nm) + g["revM"] + rm <= 128
            ):
                g["pairs"].append((la, lb))
                g["K"] += k
                g["natM"] += nm
                g["revM"] += rm
                placed = True
                break
        if not placed:
            groups.append({"pairs": [(la, lb)], "K": k, "natM": nm, "revM": rm})
    return groups


GROUPS = _pack_groups()
NG = len(GROUPS)


class GroupInfo:
    """Host-side constants for one kron group."""

    def __init__(self, g):
        self.pairs = g["pairs"]
        self.K = g["K"]
        self.krow = {}  # (la,lb) -> row offset of the pair's kron rows
        r = 0
        for la, lb in self.pairs:
            self.krow[(la, lb)] = r
            r += (2 * la + 1) * (2 * lb + 1)
        assert r == self.K
        # replication matrices [N_AROWS, K]
        self.Rs = np.zeros((N_AROWS, self.K), dtype=np.float32)
        self.Rb = np.zeros((N_AROWS, self.K), dtype=np.float32)
        for la, lb in self.pairs:
            Ka, Kb = 2 * la + 1, 2 * lb + 1
            r0 = self.krow[(la, lb)]
            for ma in range(Ka):
                for mbv in range(Kb):
                    k = r0 + ma * Kb + mbv
                    self.Rs[AROW_OF[(la, ma)], k] = 1.0
                    self.Rb[AROW_OF[(lb, mbv)], k] = 1.0
        # M columns: natural orderings, zero padding to 32-align, reversed.
        # each entry: (l1, l2, l, m, reversed) or None for padding
        self.cols = []
        for rev in (False, True):
            if rev:
                while len(self.cols) % 32:
                    self.cols.append(None)
                self.n_nat = len(self.cols)
            for la, lb in self.pairs:
                if rev and la == lb:
                    continue
                l1, l2 = (lb, la) if rev else (la, lb)
                for l in _valid_l(l1, l2):
                    for m in range(2 * l + 1):
                        self.cols.append((l1, l2, l, m, rev))
        self.M = len(self.cols)
        assert self.M <= 128
        self.n_rev = self.M - self.n_nat
        if self.n_rev == 0:
            self.n_nat = sum(1 for c in self.cols if c is not None)
            self.M = self.n_nat
            self.cols = self.cols[: self.M]
        # lhsT [K, M]
        self.lhsT = np.zeros((self.K, self.M), dtype=np.float32)
        for j, col in enumerate(self.cols):
            if col is None:
                continue
            l1, l2, l, m, rev = col
            C = CG_MATS[(l1, l2, l)]  # [2l+1, (2l1+1)(2l2+1)]
            if not rev:
                la, lb = l1, l2
                Ka, Kb = 2 * la + 1, 2 * lb + 1
                r0 = self.krow[(la, lb)]
                for ma in range(Ka):
                    for mbv in range(Kb):
                        # m1 = ma (l1 side), m2 = mbv
                        self.lhsT[r0 + ma * Kb + mbv, j] = C[m, ma * Kb + mbv]
            else:
                la, lb = l2, l1  # tile pair
                Ka, Kb = 2 * la + 1, 2 * lb + 1
                r0 = self.krow[(la, lb)]
                for ma in range(Ka):
                    for mbv in range(Kb):
                        # tile row (ma, mbv) holds A_la[ma]*A_lb[mbv];
                        # reversed kron row (m1, m2) = (mbv, ma):
                        # C index (m1)*(2*l2+1) + m2 with l2 = la
                        self.lhsT[r0 + ma * Kb + mbv, j] = C[m, mbv * Ka + ma]
        # scatter blocks: runs of cols with same (l1,l2,l,rev)
        self.blocks = []  # (col0, l1, l2, l, rev)
        j = 0
        while j < self.M:
            if self.cols[j] is None:
                j += 1
                continue
            l1, l2, l, m, rev = self.cols[j]
            assert m == 0
            self.blocks.append((j, l1, l2, l, rev))
            j += 2 * l + 1


GINFOS = [GroupInfo(g) for g in GROUPS]

# packed constants
REP_W = np.concatenate([np.concatenate([gi.Rs, gi.Rb], axis=1) for gi in GINFOS], axis=1)
REP_OFF = []
_c = 0
for gi in GINFOS:
    REP_OFF.append(_c)
    _c += 2 * gi.K
REP_COLS = _c

CW = np.zeros((128, sum(gi.M for gi in GINFOS)), dtype=np.float32)
CW_OFF = []
_c = 0
for gi in GINFOS:
    CW_OFF.append(_c)
    CW[: gi.K, _c : _c + gi.M] = gi.lhsT
    _c += gi.M
CW_COLS = _c

IDENT = np.eye(128, dtype=np.float32)

# ------------------------------------------------------------- device kernel

_COMPILED = None


def _build():
    nc = bacc.Bacc(
        "TRN2", target_bir_lowering=False, debug=False, num_devices=N_CORES
    )
    fs_d = nc.dram_tensor("fs", [B_LOC, TOTAL_IN], dt.float32, kind="ExternalInput")
    repw_d = nc.dram_tensor("repw", [N_AROWS, REP_COLS], dt.float32, kind="ExternalInput")
    cw_d = nc.dram_tensor("cw", [128, CW_COLS], dt.float32, kind="ExternalInput")
    id_d = nc.dram_tensor("ident", [128, 128], dt.float32, kind="ExternalInput")
    out_d = nc.dram_tensor(
        "out", [B_LOC, OUT_N * 2], dt.float32, kind="ExternalOutput"
    )

    with tile.TileContext(nc) as tc, ExitStack() as ctx:
        const = ctx.enter_context(tc.tile_pool(name="const", bufs=1))
        srb = ctx.enter_context(tc.tile_pool(name="srb", bufs=1))
        kron = ctx.enter_context(tc.tile_pool(name="kron", bufs=2))
        prodp = ctx.enter_context(tc.tile_pool(name="prodp", bufs=2))
        outp = ctx.enter_context(tc.tile_pool(name="outp", bufs=2))
        stp = ctx.enter_context(tc.tile_pool(name="stp", bufs=1))
        psO = ctx.enter_context(tc.tile_pool(name="psO", bufs=1, space="PSUM"))
        psT = ctx.enter_context(tc.tile_pool(name="psT", bufs=2, space="PSUM"))

        repw_s = const.tile([N_AROWS, REP_COLS], dt.float32)
        nc.sync.dma_start(repw_s[:], repw_d.ap())
        cw_s = const.tile([128, CW_COLS], dt.float32)
        nc.sync.dma_start(cw_s[:], cw_d.ap())
        id_s = const.tile([128, 128], dt.float32)
        nc.sync.dma_start(id_s[:], id_d.ap())

        # Astack [36, (b,c,t)=256]
        astack = const.tile([N_AROWS, B_LOC * 2 * TAU], dt.float32)
        for l in range(NL):
            K = K_L[l]
            for b in range(B_LOC):
                # dest [K rows at AROW_OF[(l,0)], (c=2, t=32)]
                drow = AROW_OF[(l, 0)]
                dest = astack[drow : drow + K, b * 64 : (b + 1) * 64].rearrange(
                    "p (c t) -> p c t", c=2
                )
                # source fs[b, OFF_L[l] + (t*K + m)*2 + c]
                src = fs_d.ap()[b : b + 1, OFF_L[l] : OFF_L[l] + K * TAU * 2]
                src = src.rearrange("o (t p c) -> (o p) c t", p=K, c=2)
                nc.sync.dma_start(dest, src)

        # replication matmuls -> S_g, B_g [Kg, 256]
        SB_tiles = []
        with tc.tile_pool(name="psA", bufs=1, space="PSUM") as psA:
            for gi_i, gi in enumerate(GINFOS):
                ps = psA.tile([gi.K, 256], dt.float32)
                s_t = srb.tile([gi.K, 256], dt.float32, tag=f"s{gi_i}")
                b_t = srb.tile([gi.K, 256], dt.float32, tag=f"b{gi_i}")
                o = REP_OFF[gi_i]
                nc.tensor.matmul(
                    ps[:], repw_s[:, o : o + gi.K], astack[:], start=True, stop=True
                )
                nc.scalar.copy(s_t[:], ps[:])
                ps2 = psA.tile([gi.K, 256], dt.float32)
                nc.tensor.matmul(
                    ps2[:],
                    repw_s[:, o + gi.K : o + 2 * gi.K],
                    astack[:],
                    start=True,
                    stop=True,
                )
                nc.scalar.copy(b_t[:], ps2[:])
                SB_tiles.append((s_t, b_t))

        # main loop
        for b in range(B_LOC):
            st_tiles = {}
            for l in range(NL):
                npair = len(PAIRS_OF_L[l])
                st_tiles[l] = stp.tile(
                    [128, npair * 8 * K_L[l] * 2],
                    dt.float32,
                    tag=f"st{l}",
                    name=f"st{l}_{b}",
                )
            for gi_i, gi in enumerate(GINFOS):
                s_t, b_t = SB_tiles[gi_i]
                Kg, Mg = gi.K, gi.M

                def bc(tile_, c, t_is_t1):
                    # [Kg, 32] column slice -> [Kg, 32, 32] broadcast view
                    sl = tile_[:, b * 64 + c * 32 : b * 64 + c * 32 + 32]
                    if t_is_t1:
                        # value varies with t1 (outer), broadcast over t2
                        return sl.unsqueeze(2).broadcast_to([Kg, 32, 32])
                    return sl.unsqueeze(1).broadcast_to([Kg, 32, 32])

                p_t = {}
                engines = {
                    "P1": nc.vector,
                    "P2": nc.vector,
                    "P3": nc.gpsimd,
                    "P4": nc.gpsimd,
                }
                spec = {
                    "P1": (0, 0),
                    "P2": (1, 1),
                    "P3": (0, 1),
                    "P4": (1, 0),
                }
                for nm, (ca, cb) in spec.items():
                    t_ = prodp.tile([Kg, 1024], dt.float32, tag=f"p{nm}")
                    eng = engines[nm]
                    eng.tensor_tensor(
                        t_[:].rearrange("p (a b) -> p a b", a=32),
                        bc(s_t, ca, True),
                        bc(b_t, cb, False),
                        op=mybir.AluOpType.mult,
                    )
                    p_t[nm] = t_
                kr = kron.tile([Kg, 1024], dt.float32, tag="kr")
                ki = kron.tile([Kg, 1024], dt.float32, tag="ki")
                nc.vector.tensor_tensor(
                    kr[:], p_t["P1"][:], p_t["P2"][:], op=mybir.AluOpType.subtract
                )
                nc.gpsimd.tensor_tensor(
                    ki[:], p_t["P3"][:], p_t["P4"][:], op=mybir.AluOpType.add
                )

                co = CW_OFF[gi_i]
                lhsT_nat = cw_s[:Kg, co : co + gi.n_nat]
                lhsT_rev = cw_s[:Kg, co + gi.n_nat : co + Mg]

                n_nat = gi.n_nat
                n_rev = Mg - n_nat
                for comp, ktile in (("r", kr), ("i", ki)):
                    sbo = outp.tile([Mg, 1024], dt.float32, tag="sbo")
                    for half in range(2):
                        # natural cols: free (t1,t2); this half: t1 in [16h,16h+16)
                        nat_rhs = ktile[:].rearrange("p (a b) -> p a b", a=32)[
                            :, half * 16 : (half + 1) * 16, :
                        ]
                        pso = psO.tile([n_nat, 512], dt.float32, tag="pso")
                        nc.tensor.matmul(
                            pso[:],
                            lhsT_nat,
                            nat_rhs,
                            start=True,
                            stop=True,
                        )
                        nc.scalar.copy(
                            sbo[:n_nat, half * 512 : (half + 1) * 512], pso[:]
                        )
                        if n_rev:
                            # reversed: value at (t1,t2) = tile[(t2,t1)]
                            rev_rhs = ktile[:].rearrange("p (a b) -> p b a", a=32)[
                                :, half * 16 : (half + 1) * 16, :
                            ]
                            psr = psO.tile([n_rev, 512], dt.float32, tag="psr")
                            nc.tensor.matmul(
                                psr[:],
                                lhsT_rev,
                                rev_rhs,
                                start=True,
                                stop=True,
                            )
                            nc.scalar.copy(
                                sbo[n_nat:, half * 512 : (half + 1) * 512], psr[:]
                            )
                    # 8 phase transposes: u = 8*p + r
                    pst = psT.tile([128, 1024], dt.float32, tag="pst")
                    for r in range(8):
                        src = sbo[:].rearrange("p (k r) -> p r k", r=8)[:, r, :]
                        nc.tensor.transpose(
                            pst[:, r * 128 : r * 128 + Mg],
                            src,
                            id_s[:Mg, :Mg],
                        )
                    # scatter copies into ST tiles
                    c = 0 if comp == "r" else 1
                    for bi, (col0, l1, l2, l, rev) in enumerate(gi.blocks):
                        Kl = 2 * l + 1
                        q = PAIRS_OF_L[l].index((l1, l2))
                        stt = st_tiles[l]
                        # dest [128, (r=8: step Kl*2), (m: step 2)] at q, c
                        dest = bass.AP(
                            stt[:].tensor,
                            stt[:].offset + q * 8 * Kl * 2 + c,
                            [
                                [stt[:].ap[0][0], 128],
                                [Kl * 2, 8],
                                [2, Kl],
                            ],
                        )
                        src = bass.AP(
                            pst[:].tensor,
                            pst[:].offset + col0,
                            [
                                [pst[:].ap[0][0], 128],
                                [128, 8],
                                [1, Kl],
                            ],
                        )
                        if bi % 2 == 0:
                            nc.vector.tensor_copy(dest, src)
                        else:
                            nc.scalar.copy(dest, src)
            # output DMAs
            for l in range(NL):
                Kl = 2 * l + 1
                npair = len(PAIRS_OF_L[l])
                stt = st_tiles[l]
                run = 8 * Kl * 2
                dest = bass.AP(
                    out_d.ap().tensor,
                    b * OUT_N * 2 + LBLOCK_BASE[l],
                    [
                        [run, 128],  # p: u-octet stride
                        [1024 * Kl * 2, npair],  # q
                        [1, run],
                    ],
                )
                src = bass.AP(
                    stt[:].tensor,
                    stt[:].offset,
                    [
                        [stt[:].ap[0][0], 128],
                        [run, npair],
                        [1, run],
                    ],
                )
                nc.sync.dma_start(dest, src)

    nc.compile()
    return nc


def _get_compiled():
    global _COMPILED
    if _COMPILED is None:
        _COMPILED = _build()
    return _COMPILED


def kernel(fs: np.ndarray) -> np.ndarray:
    fs = np.asarray(fs, dtype=np.float32)
    nc = _get_compiled()
    in_maps = []
    for c in range(N_CORES):
        in_maps.append(
            {
                "fs": np.ascontiguousarray(fs[c * B_LOC : (c + 1) * B_LOC]),
                "repw": REP_W,
                "cw": CW,
                "ident": IDENT,
            }
        )
    res = run_bass_kernel_spmd(nc, in_maps, list(range(N_CORES)))
    outs = [res.results[c]["out"].reshape(B_LOC, OUT_N, 2) for c in range(N_CORES)]
    return np.concatenate(outs, axis=0)


if __name__ == "__main__":
    x = np.random.default_rng(0).standard_normal((BATCH, TOTAL_IN), dtype=np.float32)
    y = kernel(fs=x)
    print("out", y.shape, y.dtype)


# revision 19
# speedup vs baseline: 58.8698x; 58.8698x over previous
"""CG sparse product (CGNet) Trainium2 kernel.

Math: for each (l, l1, l2) satisfying |l1-l2| <= l <= l1+l2 (LMAX=5),
complex Kronecker product of per-l fragments followed by Clebsch-Gordan
contraction.  Data-parallel over batch: 32 examples -> 4 per core on 8 cores.

Per-core pipeline:
  1. Strided DMAs load fs into Astack [36=(l,m), (b=4, c=2, t=32)] SBUF tile.
  2. Replication matmuls (0/1 constant weights) expand Astack rows into
     kron-row-aligned S/B tiles [Kg, 256] per group (unordered (l1<=l2) pairs
     packed into groups with sum K1*K2 <= 128, sum M <= 128).
  3. DVE/GPSIMD tensor_tensor broadcast ops form the 4 real Kronecker
     products P1=ar*br, P2=ai*bi, P3=ar*bi, P4=ai*br and kr=P1-P2, ki=P3+P4
     as [Kg, 1024=(t1,t2)] tiles.
  4. TensorE matmuls apply stacked CG matrices: psum[Mg, 1024] per comp.
     Reversed orderings (l2,l1) reuse the same kron tiles via row-permuted
     constants + (t1<->t2)-swapped access patterns.
  5. PE transposes (8 phases, u = 8*p + r) -> psum_t [128, (r=8, Mg)].
  6. Strided copies assemble final-layout staging tiles ST[b][l]
     [128, (pair, r, m, c)]; one DMA per (b, l) writes contiguous DRAM.
"""

import sys

sys.path.insert(0, "/opt/trn_rl_repo")

from contextlib import ExitStack
from math import factorial as _ft

import numpy as np

import concourse.bass as bass
import concourse.tile as tile
from concourse import bacc, mybir
from concourse.bass_utils import run_bass_kernel_spmd

dt = mybir.dt

LMAX = 5
TAU = 32
BATCH = 32
N_CORES = 8
B_LOC = BATCH // N_CORES  # 4
NL = LMAX + 1

LTUPLES = [
    (l, l1, l2)
    for l in range(NL)
    for l1 in range(NL)
    for l2 in range(NL)
    if abs(l1 - l2) <= l <= l1 + l2
]

# ---------------------------------------------------------------- CG numerics


def _cg_coef(j1, m1, j2, m2, j3, m3):
    if m1 + m2 != m3 or not (abs(j1 - j2) <= j3 <= j1 + j2):
        return 0.0
    pref = np.sqrt(
        (2 * j3 + 1)
        * _ft(j3 + j1 - j2)
        * _ft(j3 - j1 + j2)
        * _ft(j1 + j2 - j3)
        / _ft(j1 + j2 + j3 + 1)
    )
    pref *= np.sqrt(
        _ft(j3 + m3)
        * _ft(j3 - m3)
        * _ft(j1 - m1)
        * _ft(j1 + m1)
        * _ft(j2 - m2)
        * _ft(j2 + m2)
    )
    s = 0.0
    for k in range(j1 + j2 - j3 + 1):
        args = (
            k,
            j1 + j2 - j3 - k,
            j1 - m1 - k,
            j2 + m2 - k,
            j3 - j2 + m1 + k,
            j3 - j1 - m2 + k,
        )
        if min(args) < 0:
            continue
        den = 1.0
        for a in args:
            den *= _ft(a)
        s += (-1.0) ** k / den
    return pref * s


def _cg_mat(l1, l2, l):
    M = np.zeros((2 * l + 1, (2 * l1 + 1) * (2 * l2 + 1)), dtype=np.float64)
    for m in range(-l, l + 1):
        for m1 in range(-l1, l1 + 1):
            m2 = m - m1
            if abs(m2) <= l2:
                M[m + l, (m1 + l1) * (2 * l2 + 1) + (m2 + l2)] = _cg_coef(
                    l1, m1, l2, m2, l, m
                )
    return M.astype(np.float32)


CG_MATS = {(l1, l2, l): _cg_mat(l1, l2, l) for (l, l1, l2) in LTUPLES}

# ------------------------------------------------------------- layout helpers

K_L = [2 * l + 1 for l in range(NL)]
OFF_L = []  # fs column offset (elems) of each l fragment
_off = 0
for l in range(NL):
    OFF_L.append(_off)
    _off += TAU * K_L[l] * 2
TOTAL_IN = _off  # 2304

AROW = []  # Astack row for (l, m)
_r = 0
AROW_OF = {}
for l in range(NL):
    for m in range(K_L[l]):
        AROW_OF[(l, m)] = _r
        _r += 1
N_AROWS = _r  # 36

# tuple ordering / output offsets
TUP_MBASE = {}  # (l,l1,l2) -> sum of (2l'+1) over preceding tuples
_mb = 0
for t in LTUPLES:
    TUP_MBASE[t] = _mb
    _mb += 2 * t[0] + 1
M_TOTAL = _mb  # 771
OUT_N = 1024 * M_TOTAL  # per-batch rows of [.,2] output

# per-l pair lists in LTUPLES order and DRAM offsets
PAIRS_OF_L = {l: [(l1, l2) for (ll, l1, l2) in LTUPLES if ll == l] for l in range(NL)}
LBLOCK_BASE = {}  # l -> flat f32-elem offset of l block within one batch row
_fb = 0
for l in range(NL):
    LBLOCK_BASE[l] = _fb
    _fb += len(PAIRS_OF_L[l]) * 1024 * K_L[l] * 2
assert _fb == OUT_N * 2

# ------------------------------------------------------- group packing (host)

UNORD = [(la, lb) for la in range(NL) for lb in range(la, NL)]


def _valid_l(l1, l2):
    return [l for l in range(NL) if abs(l1 - l2) <= l <= l1 + l2]


def _pair_m(la, lb):
    m = sum(2 * l + 1 for l in _valid_l(la, lb))
    if la != lb:
        m *= 2
    return m


def _nat_m(la, lb):
    return sum(2 * l + 1 for l in _valid_l(la, lb))


def _rev_m(la, lb):
    return 0 if la == lb else _nat_m(la, lb)


def _ceil32(x):
    return (x + 31) // 32 * 32


def _pack_groups():
    items = sorted(UNORD, key=lambda p: -(2 * p[0] + 1) * (2 * p[1] + 1))
    groups = []  # list of dicts {pairs, K, natM, revM}
    for la, lb in items:
        k = (2 * la + 1) * (2 * lb + 1)
        nm, rm = _nat_m(la, lb), _rev_m(la, lb)
        placed = False
        for g in groups:
            if (
                g["K"] + k <= 128
                and _ceil32(g["natM"] + nm) + g["revM"] + rm <= 128
            ):
                g["pairs"].append((la, lb))
                g["K"] += k
                g["natM"] += nm
                g["revM"] += rm
                placed = True
                break
        if not placed:
            groups.append({"pairs": [(la, lb)], "K": k, "natM": nm, "revM": rm})
    return groups


GROUPS = _pack_groups()
NG = len(GROUPS)


class GroupInfo:
    """Host-side constants for one kron group."""

    def __init__(self, g):
        self.pairs = g["pairs"]
        self.K = g["K"]
        self.krow = {}  # (la,lb) -> row offset of the pair's kron rows
        r = 0
        for la, lb in self.pairs:
            self.krow[(la, lb)] = r
            r += (2 * la + 1) * (2 * lb + 1)
        assert r == self.K
        # replication matrices [N_AROWS, K]
        self.Rs = np.zeros((N_AROWS, self.K), dtype=np.float32)
        self.Rb = np.zeros((N_AROWS, self.K), dtype=np.float32)
        for la, lb in self.pairs:
            Ka, Kb = 2 * la + 1, 2 * lb + 1
            r0 = self.krow[(la, lb)]
            for ma in range(Ka):
                for mbv in range(Kb):
                    k = r0 + ma * Kb + mbv
                    self.Rs[AROW_OF[(la, ma)], k] = 1.0
                    self.Rb[AROW_OF[(lb, mbv)], k] = 1.0
        # M columns: natural orderings, zero padding to 32-align, reversed.
        # each entry: (l1, l2, l, m, reversed) or None for padding
        self.cols = []
        for rev in (False, True):
            if rev:
                while len(self.cols) % 32:
                    self.cols.append(None)
                self.n_nat = len(self.cols)
            for la, lb in self.pairs:
                if rev and la == lb:
                    continue
                l1, l2 = (lb, la) if rev else (la, lb)
                for l in _valid_l(l1, l2):
                    for m in range(2 * l + 1):
                        self.cols.append((l1, l2, l, m, rev))
        self.M = len(self.cols)
        assert self.M <= 128
        self.n_rev = self.M - self.n_nat
        if self.n_rev == 0:
            self.n_nat = sum(1 for c in self.cols if c is not None)
            self.M = self.n_nat
            self.cols = self.cols[: self.M]
        # lhsT [K, M]
        self.lhsT = np.zeros((self.K, self.M), dtype=np.float32)
        for j, col in enumerate(self.cols):
            if col is None:
                continue
            l1, l2, l, m, rev = col
            C = CG_MATS[(l1, l2, l)]  # [2l+1, (2l1+1)(2l2+1)]
            if not rev:
                la, lb = l1, l2
                Ka, Kb = 2 * la + 1, 2 * lb + 1
                r0 = self.krow[(la, lb)]
                for ma in range(Ka):
                    for mbv in range(Kb):
                        # m1 = ma (l1 side), m2 = mbv
                        self.lhsT[r0 + ma * Kb + mbv, j] = C[m, ma * Kb + mbv]
            else:
                la, lb = l2, l1  # tile pair
                Ka, Kb = 2 * la + 1, 2 * lb + 1
                r0 = self.krow[(la, lb)]
                for ma in range(Ka):
                    for mbv in range(Kb):
                        # tile row (ma, mbv) holds A_la[ma]*A_lb[mbv];
                        # reversed kron row (m1, m2) = (mbv, ma):
                        # C index (m1)*(2*l2+1) + m2 with l2 = la
                        self.lhsT[r0 + ma * Kb + mbv, j] = C[m, mbv * Ka + ma]
        # scatter blocks: runs of cols with same (l1,l2,l,rev)
        self.blocks = []  # (col0, l1, l2, l, rev)
        j = 0
        while j < self.M:
            if self.cols[j] is None:
                j += 1
                continue
            l1, l2, l, m, rev = self.cols[j]
            assert m == 0
            self.blocks.append((j, l1, l2, l, rev))
            j += 2 * l + 1


GINFOS = [GroupInfo(g) for g in GROUPS]

# packed constants
REP_W = np.concatenate([np.concatenate([gi.Rs, gi.Rb], axis=1) for gi in GINFOS], axis=1)
REP_OFF = []
_c = 0
for gi in GINFOS:
    REP_OFF.append(_c)
    _c += 2 * gi.K
REP_COLS = _c

CW = np.zeros((128, sum(gi.M for gi in GINFOS)), dtype=np.float32)
CW_OFF = []
_c = 0
for gi in GINFOS:
    CW_OFF.append(_c)
    CW[: gi.K, _c : _c + gi.M] = gi.lhsT
    _c += gi.M
CW_COLS = _c

IDENT = np.eye(128, dtype=np.float32)

# ------------------------------------------------------------- device kernel

_COMPILED = None


def _build():
    nc = bacc.Bacc(
        "TRN2", target_bir_lowering=False, debug=False, num_devices=N_CORES
    )
    fs_d = nc.dram_tensor("fs", [B_LOC, TOTAL_IN], dt.float32, kind="ExternalInput")
    repw_d = nc.dram_tensor("repw", [N_AROWS, REP_COLS], dt.float32, kind="ExternalInput")
    cw_d = nc.dram_tensor("cw", [128, CW_COLS], dt.float32, kind="ExternalInput")
    id_d = nc.dram_tensor("ident", [128, 128], dt.float32, kind="ExternalInput")
    out_d = nc.dram_tensor(
        "out", [B_LOC, OUT_N * 2], dt.float32, kind="ExternalOutput"
    )

    with tile.TileContext(nc) as tc, ExitStack() as ctx:
        const = ctx.enter_context(tc.tile_pool(name="const", bufs=1))
        srb = ctx.enter_context(tc.tile_pool(name="srb", bufs=1))
        kron = ctx.enter_context(tc.tile_pool(name="kron", bufs=2))
        prodp = ctx.enter_context(tc.tile_pool(name="prodp", bufs=2))
        outp = ctx.enter_context(tc.tile_pool(name="outp", bufs=2))
        stp = ctx.enter_context(tc.tile_pool(name="stp", bufs=1))
        psO = ctx.enter_context(tc.tile_pool(name="psO", bufs=1, space="PSUM"))
        psT = ctx.enter_context(tc.tile_pool(name="psT", bufs=2, space="PSUM"))

        repw_s = const.tile([N_AROWS, REP_COLS], dt.float32)
        nc.sync.dma_start(repw_s[:], repw_d.ap())
        cw_s = const.tile([128, CW_COLS], dt.float32)
        nc.sync.dma_start(cw_s[:], cw_d.ap())
        id_s = const.tile([128, 128], dt.float32)
        nc.sync.dma_start(id_s[:], id_d.ap())

        # Astack [36, (b,c,t)=256]
        astack = const.tile([N_AROWS, B_LOC * 2 * TAU], dt.float32)
        for l in range(NL):
            K = K_L[l]
            for b in range(B_LOC):
                # dest [K rows at AROW_OF[(l,0)], (c=2, t=32)]
                drow = AROW_OF[(l, 0)]
                dest = astack[drow : drow + K, b * 64 : (b + 1) * 64].rearrange(
                    "p (c t) -> p c t", c=2
                )
                # source fs[b, OFF_L[l] + (t*K + m)*2 + c]
                src = fs_d.ap()[b : b + 1, OFF_L[l] : OFF_L[l] + K * TAU * 2]
                src = src.rearrange("o (t p c) -> (o p) c t", p=K, c=2)
                nc.sync.dma_start(dest, src)

        # replication matmuls -> S_g, B_g [Kg, 256]
        SB_tiles = []
        with tc.tile_pool(name="psA", bufs=1, space="PSUM") as psA:
            for gi_i, gi in enumerate(GINFOS):
                ps = psA.tile([gi.K, 256], dt.float32)
                s_t = srb.tile([gi.K, 256], dt.float32, tag=f"s{gi_i}")
                b_t = srb.tile([gi.K, 256], dt.float32, tag=f"b{gi_i}")
                o = REP_OFF[gi_i]
                nc.tensor.matmul(
                    ps[:], repw_s[:, o : o + gi.K], astack[:], start=True, stop=True
                )
                nc.scalar.copy(s_t[:], ps[:])
                ps2 = psA.tile([gi.K, 256], dt.float32)
                nc.tensor.matmul(
                    ps2[:],
                    repw_s[:, o + gi.K : o + 2 * gi.K],
                    astack[:],
                    start=True,
                    stop=True,
                )
                nc.scalar.copy(b_t[:], ps2[:])
                SB_tiles.append((s_t, b_t))

        # main loop
        for b in range(B_LOC):
            st_tiles = {}
            for l in range(NL):
                npair = len(PAIRS_OF_L[l])
                st_tiles[l] = stp.tile(
                    [128, npair * 8 * K_L[l] * 2],
                    dt.float32,
                    tag=f"st{l}",
                    name=f"st{l}_{b}",
                )
            for gi_i, gi in enumerate(GINFOS):
                s_t, b_t = SB_tiles[gi_i]
                Kg, Mg = gi.K, gi.M

                def bc(tile_, c, t_is_t1):
                    # [Kg, 32] column slice -> [Kg, 32, 32] broadcast view
                    sl = tile_[:, b * 64 + c * 32 : b * 64 + c * 32 + 32]
                    if t_is_t1:
                        # value varies with t1 (outer), broadcast over t2
                        return sl.unsqueeze(2).broadcast_to([Kg, 32, 32])
                    return sl.unsqueeze(1).broadcast_to([Kg, 32, 32])

                p_t = {}
                engines = {
                    "P1": nc.vector,
                    "P2": nc.vector,
                    "P3": nc.gpsimd,
                    "P4": nc.gpsimd,
                }
                spec = {
                    "P1": (0, 0),
                    "P2": (1, 1),
                    "P3": (0, 1),
                    "P4": (1, 0),
                }
                for nm, (ca, cb) in spec.items():
                    t_ = prodp.tile([Kg, 1024], dt.float32, tag=f"p{nm}")
                    eng = engines[nm]
                    eng.tensor_tensor(
                        t_[:].rearrange("p (a b) -> p a b", a=32),
                        bc(s_t, ca, True),
                        bc(b_t, cb, False),
                        op=mybir.AluOpType.mult,
                    )
                    p_t[nm] = t_
                kr = kron.tile([Kg, 1024], dt.float32, tag="kr")
                ki = kron.tile([Kg, 1024], dt.float32, tag="ki")
                nc.vector.tensor_tensor(
                    kr[:], p_t["P1"][:], p_t["P2"][:], op=mybir.AluOpType.subtract
                )
                nc.gpsimd.tensor_tensor(
                    ki[:], p_t["P3"][:], p_t["P4"][:], op=mybir.AluOpType.add
                )

                co = CW_OFF[gi_i]
                lhsT_nat = cw_s[:Kg, co : co + gi.n_nat]
                lhsT_rev = cw_s[:Kg, co + gi.n_nat : co + Mg]

                n_nat = gi.n_nat
                n_rev = Mg - n_nat
                for comp, ktile in (("r", kr), ("i", ki)):
                    sbo = outp.tile([Mg, 1024], dt.float32, tag="sbo")
                    for half in range(2):
                        # natural cols: free (t1,t2); this half: t1 in [16h,16h+16)
                        nat_rhs = ktile[:].rearrange("p (a b) -> p a b", a=32)[
                            :, half * 16 : (half + 1) * 16, :
                        ]
                        pso = psO.tile([n_nat, 512], dt.float32, tag="pso")
                        nc.tensor.matmul(
                            pso[:],
                            lhsT_nat,
                            nat_rhs,
                            start=True,
                            stop=True,
                        )
                        nc.scalar.copy(
                            sbo[:n_nat, half * 512 : (half + 1) * 512], pso[:]
                        )
                        if n_rev:
                            # reversed: value at (t1,t2) = tile[(t2,t1)]
                            rev_rhs = ktile[:].rearrange("p (a b) -> p b a", a=32)[
                                :, half * 16 : (half + 1) * 16, :
                            ]
                            psr = psO.tile([n_rev, 512], dt.float32, tag="psr")
                            nc.tensor.matmul(
                                psr[:],
                                lhsT_rev,
                                rev_rhs,
                                start=True,
                                stop=True,
                            )
                            nc.scalar.copy(
                                sbo[n_nat:, half * 512 : (half + 1) * 512], psr[:]
                            )
                    # 8 phase transposes: u = 8*p + r
                    pst = psT.tile([128, 1024], dt.float32, tag="pst")
                    for r in range(8):
                        src = sbo[:].rearrange("p (k r) -> p r k", r=8)[:, r, :]
                        nc.tensor.transpose(
                            pst[:, r * 128 : r * 128 + Mg],
                            src,
                            id_s[:Mg, :Mg],
                        )
                    # scatter copies into ST tiles
                    c = 0 if comp == "r" else 1
                    for bi, (col0, l1, l2, l, rev) in enumerate(gi.blocks):
                        Kl = 2 * l + 1
                        q = PAIRS_OF_L[l].index((l1, l2))
                        stt = st_tiles[l]
                        # dest [128, (r=8: step Kl*2), (m: step 2)] at q, c
                        dest = bass.AP(
                            stt[:].tensor,
                            stt[:].offset + q * 8 * Kl * 2 + c,
                            [
                                [stt[:].ap[0][0], 128],
                                [Kl * 2, 8],
                                [2, Kl],
                            ],
                        )
                        src = bass.AP(
                            pst[:].tensor,
                            pst[:].offset + col0,
                            [
                                [pst[:].ap[0][0], 128],
                                [128, 8],
                                [1, Kl],
                            ],
                        )
                        if bi % 2 == 0:
                            nc.vector.tensor_copy(dest, src)
                        else:
                            nc.scalar.copy(dest, src)
            # output DMAs
            for l in range(NL):
                Kl = 2 * l + 1
                npair = len(PAIRS_OF_L[l])
                stt = st_tiles[l]
                run = 8 * Kl * 2
                dest = bass.AP(
                    out_d.ap().tensor,
                    b * OUT_N * 2 + LBLOCK_BASE[l],
                    [
                        [run, 128],  # p: u-octet stride
                        [1024 * Kl * 2, npair],  # q
                        [1, run],
                    ],
                )
                src = bass.AP(
                    stt[:].tensor,
                    stt[:].offset,
                    [
                        [stt[:].ap[0][0], 128],
                        [run, npair],
                        [1, run],
                    ],
                )
                nc.sync.dma_start(dest, src)

    nc.compile()
    return nc


def _get_compiled():
    global _COMPILED
    if _COMPILED is None:
        _COMPILED = _build()
    return _COMPILED


_EXEC = None


def _get_exec():
    """Build the sharded PJRT executable once (mirrors bass2jax.run_bass_via_pjrt)."""
    global _EXEC
    if _EXEC is not None:
        return _EXEC
    import jax
    import jax.numpy as jnp
    from jax.sharding import Mesh, PartitionSpec
    from jax.experimental.shard_map import shard_map
    from concourse import bass2jax, mybir as _mybir

    bass2jax.install_neuronx_cc_hook()
    nc = _get_compiled()

    part_name = nc.partition_id_tensor.name if nc.partition_id_tensor else None
    in_names, out_names, out_avals = [], [], []
    for alloc in nc.m.functions[0].allocations:
        if not isinstance(alloc, _mybir.MemoryLocationSet):
            continue
        name = alloc.memorylocations[0].name
        if alloc.kind == "ExternalInput":
            if name != part_name:
                in_names.append(name)
        elif alloc.kind == "ExternalOutput":
            shape = tuple(alloc.tensor_shape)
            out_avals.append(jax.core.ShapedArray(shape, _mybir.dt.np(alloc.dtype)))
            out_names.append(name)
    n_params = len(in_names)
    n_outs = len(out_avals)
    all_in_names = in_names + out_names
    if part_name is not None:
        all_in_names.append(part_name)

    def _body(*args):
        operands = list(args)
        if part_name is not None:
            operands.append(bass2jax.partition_id_tensor())
        outs = bass2jax._bass_exec_p.bind(
            *operands,
            out_avals=tuple(out_avals),
            in_names=tuple(all_in_names),
            out_names=tuple(out_names),
            lowering_input_output_aliases=(),
            sim_require_finite=True,
            sim_require_nnan=True,
            nc=nc,
        )
        return tuple(outs)

    devices = jax.devices()[:N_CORES]
    mesh = Mesh(np.asarray(devices), ("core",))
    donate = tuple(range(n_params, n_params + n_outs))
    sharded = jax.jit(
        shard_map(
            _body,
            mesh=mesh,
            in_specs=(PartitionSpec("core"),) * (n_params + n_outs),
            out_specs=(PartitionSpec("core"),) * n_outs,
            check_rep=False,
        ),
        donate_argnums=donate,
        keep_unused=True,
    )
    # device-resident zero output buffers + on-device cloner (avoids a
    # 200MB host->device upload per call; donation consumes the clone)
    zeros_host = [
        np.zeros((N_CORES * a.shape[0], *a.shape[1:]), a.dtype) for a in out_avals
    ]
    zspec = jax.sharding.NamedSharding(mesh, PartitionSpec("core"))
    zeros_dev = [jax.device_put(z, zspec) for z in zeros_host]
    clone = jax.jit(lambda *zs: tuple(jnp.copy(z) for z in zs))

    _EXEC = (sharded, clone, zeros_dev, in_names, out_names, out_avals, zspec)
    return _EXEC


def _run_device(fs: np.ndarray):
    """Upload inputs, run the sharded executable; returns device arrays."""
    import jax

    sharded, clone, zeros_dev, in_names, out_names, out_avals, zspec = _get_exec()
    per_core = {
        "fs": fs.reshape(N_CORES, B_LOC, TOTAL_IN),
        "repw": np.broadcast_to(REP_W, (N_CORES, *REP_W.shape)),
        "cw": np.broadcast_to(CW, (N_CORES, *CW.shape)),
        "ident": np.broadcast_to(IDENT, (N_CORES, *IDENT.shape)),
    }
    ins = [
        np.ascontiguousarray(per_core[n].reshape(-1, *per_core[n].shape[2:]))
        for n in in_names
    ]
    zcopies = clone(*zeros_dev)
    outs = sharded(*ins, *zcopies)
    return outs, out_names, out_avals


def kernel(fs: np.ndarray) -> np.ndarray:
    fs = np.asarray(fs, dtype=np.float32)
    outs, out_names, out_avals = _run_device(fs)
    out = np.asarray(outs[out_names.index("out")])
    return out.reshape(BATCH, OUT_N, 2)


if __name__ == "__main__":
    x = np.random.default_rng(0).standard_normal((BATCH, TOTAL_IN), dtype=np.float32)
    y = kernel(fs=x)
    print("out", y.shape, y.dtype)


# revision 20
# speedup vs baseline: 17054.2465x; 289.6943x over previous
"""CG sparse product (CGNet) Trainium2 kernel.

Math: for each (l, l1, l2) satisfying |l1-l2| <= l <= l1+l2 (LMAX=5),
complex Kronecker product of per-l fragments followed by Clebsch-Gordan
contraction.  Data-parallel over batch: 32 examples -> 4 per core on 8 cores.

Per-core pipeline:
  1. Strided DMAs load fs into Astack [36=(l,m), (b=4, c=2, t=32)] SBUF tile.
  2. Replication matmuls (0/1 constant weights) expand Astack rows into
     kron-row-aligned S/B tiles [Kg, 256] per group (unordered (l1<=l2) pairs
     packed into groups with sum K1*K2 <= 128, sum M <= 128).
  3. DVE/GPSIMD tensor_tensor broadcast ops form the 4 real Kronecker
     products P1=ar*br, P2=ai*bi, P3=ar*bi, P4=ai*br and kr=P1-P2, ki=P3+P4
     as [Kg, 1024=(t1,t2)] tiles.
  4. TensorE matmuls apply stacked CG matrices: psum[Mg, 1024] per comp.
     Reversed orderings (l2,l1) reuse the same kron tiles via row-permuted
     constants + (t1<->t2)-swapped access patterns.
  5. PE transposes (8 phases, u = 8*p + r) -> psum_t [128, (r=8, Mg)].
  6. Strided copies assemble final-layout staging tiles ST[b][l]
     [128, (pair, r, m, c)]; one DMA per (b, l) writes contiguous DRAM.
"""

import sys

sys.path.insert(0, "/opt/trn_rl_repo")

from contextlib import ExitStack
from math import factorial as _ft

import numpy as np

import concourse.bass as bass
import concourse.tile as tile
from concourse import bacc, mybir
from concourse.bass_utils import run_bass_kernel_spmd

dt = mybir.dt

LMAX = 5
TAU = 32
BATCH = 32
N_CORES = 8
B_LOC = BATCH // N_CORES  # 4
NL = LMAX + 1

LTUPLES = [
    (l, l1, l2)
    for l in range(NL)
    for l1 in range(NL)
    for l2 in range(NL)
    if abs(l1 - l2) <= l <= l1 + l2
]

# ---------------------------------------------------------------- CG numerics


def _cg_coef(j1, m1, j2, m2, j3, m3):
    if m1 + m2 != m3 or not (abs(j1 - j2) <= j3 <= j1 + j2):
        return 0.0
    pref = np.sqrt(
        (2 * j3 + 1)
        * _ft(j3 + j1 - j2)
        * _ft(j3 - j1 + j2)
        * _ft(j1 + j2 - j3)
        / _ft(j1 + j2 + j3 + 1)
    )
    pref *= np.sqrt(
        _ft(j3 + m3)
        * _ft(j3 - m3)
        * _ft(j1 - m1)
        * _ft(j1 + m1)
        * _ft(j2 - m2)
        * _ft(j2 + m2)
    )
    s = 0.0
    for k in range(j1 + j2 - j3 + 1):
        args = (
            k,
            j1 + j2 - j3 - k,
            j1 - m1 - k,
            j2 + m2 - k,
            j3 - j2 + m1 + k,
            j3 - j1 - m2 + k,
        )
        if min(args) < 0:
            continue
        den = 1.0
        for a in args:
            den *= _ft(a)
        s += (-1.0) ** k / den
    return pref * s


def _cg_mat(l1, l2, l):
    M = np.zeros((2 * l + 1, (2 * l1 + 1) * (2 * l2 + 1)), dtype=np.float64)
    for m in range(-l, l + 1):
        for m1 in range(-l1, l1 + 1):
            m2 = m - m1
            if abs(m2) <= l2:
                M[m + l, (m1 + l1) * (2 * l2 + 1) + (m2 + l2)] = _cg_coef(
                    l1, m1, l2, m2, l, m
                )
    return M.astype(np.float32)


CG_MATS = {(l1, l2, l): _cg_mat(l1, l2, l) for (l, l1, l2) in LTUPLES}

# ------------------------------------------------------------- layout helpers

K_L = [2 * l + 1 for l in range(NL)]
OFF_L = []  # fs column offset (elems) of each l fragment
_off = 0
for l in range(NL):
    OFF_L.append(_off)
    _off += TAU * K_L[l] * 2
TOTAL_IN = _off  # 2304

AROW = []  # Astack row for (l, m)
_r = 0
AROW_OF = {}
for l in range(NL):
    for m in range(K_L[l]):
        AROW_OF[(l, m)] = _r
        _r += 1
N_AROWS = _r  # 36

# tuple ordering / output offsets
TUP_MBASE = {}  # (l,l1,l2) -> sum of (2l'+1) over preceding tuples
_mb = 0
for t in LTUPLES:
    TUP_MBASE[t] = _mb
    _mb += 2 * t[0] + 1
M_TOTAL = _mb  # 771
OUT_N = 1024 * M_TOTAL  # per-batch rows of [.,2] output

# per-l pair lists in LTUPLES order and DRAM offsets
PAIRS_OF_L = {l: [(l1, l2) for (ll, l1, l2) in LTUPLES if ll == l] for l in range(NL)}
LBLOCK_BASE = {}  # l -> flat f32-elem offset of l block within one batch row
_fb = 0
for l in range(NL):
    LBLOCK_BASE[l] = _fb
    _fb += len(PAIRS_OF_L[l]) * 1024 * K_L[l] * 2
assert _fb == OUT_N * 2

# ------------------------------------------------------- group packing (host)

UNORD = [(la, lb) for la in range(NL) for lb in range(la, NL)]


def _valid_l(l1, l2):
    return [l for l in range(NL) if abs(l1 - l2) <= l <= l1 + l2]


def _pair_m(la, lb):
    m = sum(2 * l + 1 for l in _valid_l(la, lb))
    if la != lb:
        m *= 2
    return m


def _nat_m(la, lb):
    return sum(2 * l + 1 for l in _valid_l(la, lb))


def _rev_m(la, lb):
    return 0 if la == lb else _nat_m(la, lb)


def _ceil32(x):
    return (x + 31) // 32 * 32


def _pack_groups():
    items = sorted(UNORD, key=lambda p: -(2 * p[0] + 1) * (2 * p[1] + 1))
    groups = []  # list of dicts {pairs, K, natM, revM}
    for la, lb in items:
        k = (2 * la + 1) * (2 * lb + 1)
        nm, rm = _nat_m(la, lb), _rev_m(la, lb)
        placed = False
        for g in groups:
            if (
                g["K"] + k <= 128
                and _ceil32(g["natM"] + nm) + g["revM"] + rm <= 128
            ):
                g["pairs"].append((la, lb))
                g["K"] += k
                g["natM"] += nm
                g["revM"] += rm
                placed = True
                break
        if not placed:
            groups.append({"pairs": [(la, lb)], "K": k, "natM": nm, "revM": rm})
    return groups


GROUPS = _pack_groups()
NG = len(GROUPS)


class GroupInfo:
    """Host-side constants for one kron group."""

    def __init__(self, g):
        self.pairs = g["pairs"]
        self.K = g["K"]
        self.krow = {}  # (la,lb) -> row offset of the pair's kron rows
        r = 0
        for la, lb in self.pairs:
            self.krow[(la, lb)] = r
            r += (2 * la + 1) * (2 * lb + 1)
        assert r == self.K
        # replication matrices [N_AROWS, K]
        self.Rs = np.zeros((N_AROWS, self.K), dtype=np.float32)
        self.Rb = np.zeros((N_AROWS, self.K), dtype=np.float32)
        for la, lb in self.pairs:
            Ka, Kb = 2 * la + 1, 2 * lb + 1
            r0 = self.krow[(la, lb)]
            for ma in range(Ka):
                for mbv in range(Kb):
                    k = r0 + ma * Kb + mbv
                    self.Rs[AROW_OF[(la, ma)], k] = 1.0
                    self.Rb[AROW_OF[(lb, mbv)], k] = 1.0
        # M columns: natural orderings, zero padding to 32-align, reversed.
        # each entry: (l1, l2, l, m, reversed) or None for padding
        self.cols = []
        for rev in (False, True):
            if rev:
                while len(self.cols) % 32:
                    self.cols.append(None)
                self.n_nat = len(self.cols)
            for la, lb in self.pairs:
                if rev and la == lb:
                    continue
                l1, l2 = (lb, la) if rev else (la, lb)
                for l in _valid_l(l1, l2):
                    for m in range(2 * l + 1):
                        self.cols.append((l1, l2, l, m, rev))
        self.M = len(self.cols)
        assert self.M <= 128
        self.n_rev = self.M - self.n_nat
        if self.n_rev == 0:
            self.n_nat = sum(1 for c in self.cols if c is not None)
            self.M = self.n_nat
            self.cols = self.cols[: self.M]
        # lhsT [K, M]
        self.lhsT = np.zeros((self.K, self.M), dtype=np.float32)
        for j, col in enumerate(self.cols):
            if col is None:
                continue
            l1, l2, l, m, rev = col
            C = CG_MATS[(l1, l2, l)]  # [2l+1, (2l1+1)(2l2+1)]
            if not rev:
                la, lb = l1, l2
                Ka, Kb = 2 * la + 1, 2 * lb + 1
                r0 = self.krow[(la, lb)]
                for ma in range(Ka):
                    for mbv in range(Kb):
                        # m1 = ma (l1 side), m2 = mbv
                        self.lhsT[r0 + ma * Kb + mbv, j] = C[m, ma * Kb + mbv]
            else:
                la, lb = l2, l1  # tile pair
                Ka, Kb = 2 * la + 1, 2 * lb + 1
                r0 = self.krow[(la, lb)]
                for ma in range(Ka):
                    for mbv in range(Kb):
                        # tile row (ma, mbv) holds A_la[ma]*A_lb[mbv];
                        # reversed kron row (m1, m2) = (mbv, ma):
                        # C index (m1)*(2*l2+1) + m2 with l2 = la
                        self.lhsT[r0 + ma * Kb + mbv, j] = C[m, mbv * Ka + ma]
        # scatter blocks: runs of cols with same (l1,l2,l,rev)
        self.blocks = []  # (col0, l1, l2, l, rev)
        j = 0
        while j < self.M:
            if self.cols[j] is None:
                j += 1
                continue
            l1, l2, l, m, rev = self.cols[j]
            assert m == 0
            self.blocks.append((j, l1, l2, l, rev))
            j += 2 * l + 1


GINFOS = [GroupInfo(g) for g in GROUPS]

# packed constants
REP_W = np.concatenate([np.concatenate([gi.Rs, gi.Rb], axis=1) for gi in GINFOS], axis=1)
REP_OFF = []
_c = 0
for gi in GINFOS:
    REP_OFF.append(_c)
    _c += 2 * gi.K
REP_COLS = _c

CW = np.zeros((128, sum(gi.M for gi in GINFOS)), dtype=np.float32)
CW_OFF = []
_c = 0
for gi in GINFOS:
    CW_OFF.append(_c)
    CW[: gi.K, _c : _c + gi.M] = gi.lhsT
    _c += gi.M
CW_COLS = _c

IDENT = np.eye(128, dtype=np.float32)

# ------------------------------------------------------------- device kernel

_COMPILED = None


REPS = 1  # number of times the device body runs (for slope timing); set via set_reps()


def set_reps(r):
    global REPS, _COMPILED, _EXEC
    REPS = r
    _COMPILED = None
    _EXEC = None


def _build():
    nc = bacc.Bacc(
        "TRN2", target_bir_lowering=False, debug=False, num_devices=N_CORES
    )
    fs_d = nc.dram_tensor("fs", [B_LOC, TOTAL_IN], dt.float32, kind="ExternalInput")
    repw_d = nc.dram_tensor("repw", [N_AROWS, REP_COLS], dt.float32, kind="ExternalInput")
    cw_d = nc.dram_tensor("cw", [128, CW_COLS], dt.float32, kind="ExternalInput")
    id_d = nc.dram_tensor("ident", [128, 128], dt.float32, kind="ExternalInput")
    out_d = nc.dram_tensor(
        "out", [B_LOC, OUT_N * 2], dt.float32, kind="ExternalOutput"
    )

    with tile.TileContext(nc) as tc, ExitStack() as ctx:
        const = ctx.enter_context(tc.tile_pool(name="const", bufs=1))
        srb = ctx.enter_context(tc.tile_pool(name="srb", bufs=1))
        kron = ctx.enter_context(tc.tile_pool(name="kron", bufs=2))
        prodp = ctx.enter_context(tc.tile_pool(name="prodp", bufs=2))
        outp = ctx.enter_context(tc.tile_pool(name="outp", bufs=2))
        stp = ctx.enter_context(tc.tile_pool(name="stp", bufs=1))
        psO = ctx.enter_context(tc.tile_pool(name="psO", bufs=1, space="PSUM"))
        psT = ctx.enter_context(tc.tile_pool(name="psT", bufs=2, space="PSUM"))

        repw_s = const.tile([N_AROWS, REP_COLS], dt.float32)
        nc.sync.dma_start(repw_s[:], repw_d.ap())
        cw_s = const.tile([128, CW_COLS], dt.float32)
        nc.sync.dma_start(cw_s[:], cw_d.ap())
        id_s = const.tile([128, 128], dt.float32)
        nc.sync.dma_start(id_s[:], id_d.ap())

        # Astack [36, (b,c,t)=256]
        astack = const.tile([N_AROWS, B_LOC * 2 * TAU], dt.float32)
        for l in range(NL):
            K = K_L[l]
            for b in range(B_LOC):
                # dest [K rows at AROW_OF[(l,0)], (c=2, t=32)]
                drow = AROW_OF[(l, 0)]
                dest = astack[drow : drow + K, b * 64 : (b + 1) * 64].rearrange(
                    "p (c t) -> p c t", c=2
                )
                # source fs[b, OFF_L[l] + (t*K + m)*2 + c]
                src = fs_d.ap()[b : b + 1, OFF_L[l] : OFF_L[l] + K * TAU * 2]
                src = src.rearrange("o (t p c) -> (o p) c t", p=K, c=2)
                nc.sync.dma_start(dest, src)

        # replication matmuls -> S_g, B_g [Kg, 256]
        SB_tiles = []
        with tc.tile_pool(name="psA", bufs=1, space="PSUM") as psA:
            for gi_i, gi in enumerate(GINFOS):
                ps = psA.tile([gi.K, 256], dt.float32)
                s_t = srb.tile([gi.K, 256], dt.float32, tag=f"s{gi_i}")
                b_t = srb.tile([gi.K, 256], dt.float32, tag=f"b{gi_i}")
                o = REP_OFF[gi_i]
                nc.tensor.matmul(
                    ps[:], repw_s[:, o : o + gi.K], astack[:], start=True, stop=True
                )
                nc.scalar.copy(s_t[:], ps[:])
                ps2 = psA.tile([gi.K, 256], dt.float32)
                nc.tensor.matmul(
                    ps2[:],
                    repw_s[:, o + gi.K : o + 2 * gi.K],
                    astack[:],
                    start=True,
                    stop=True,
                )
                nc.scalar.copy(b_t[:], ps2[:])
                SB_tiles.append((s_t, b_t))

        # main loop
        for _rep in range(REPS):
          for b in range(B_LOC):
            st_tiles = {}
            for l in range(NL):
                npair = len(PAIRS_OF_L[l])
                st_tiles[l] = stp.tile(
                    [128, npair * 8 * K_L[l] * 2],
                    dt.float32,
                    tag=f"st{l}",
                    name=f"st{l}_{b}",
                )
            for gi_i, gi in enumerate(GINFOS):
                s_t, b_t = SB_tiles[gi_i]
                Kg, Mg = gi.K, gi.M

                def bc(tile_, c, t_is_t1):
                    # [Kg, 32] column slice -> [Kg, 32, 32] broadcast view
                    sl = tile_[:, b * 64 + c * 32 : b * 64 + c * 32 + 32]
                    if t_is_t1:
                        # value varies with t1 (outer), broadcast over t2
                        return sl.unsqueeze(2).broadcast_to([Kg, 32, 32])
                    return sl.unsqueeze(1).broadcast_to([Kg, 32, 32])

                p_t = {}
                engines = {
                    "P1": nc.vector,
                    "P2": nc.vector,
                    "P3": nc.gpsimd,
                    "P4": nc.gpsimd,
                }
                spec = {
                    "P1": (0, 0),
                    "P2": (1, 1),
                    "P3": (0, 1),
                    "P4": (1, 0),
                }
                for nm, (ca, cb) in spec.items():
                    t_ = prodp.tile([Kg, 1024], dt.float32, tag=f"p{nm}")
                    eng = engines[nm]
                    eng.tensor_tensor(
                        t_[:].rearrange("p (a b) -> p a b", a=32),
                        bc(s_t, ca, True),
                        bc(b_t, cb, False),
                        op=mybir.AluOpType.mult,
                    )
                    p_t[nm] = t_
                kr = kron.tile([Kg, 1024], dt.float32, tag="kr")
                ki = kron.tile([Kg, 1024], dt.float32, tag="ki")
                nc.vector.tensor_tensor(
                    kr[:], p_t["P1"][:], p_t["P2"][:], op=mybir.AluOpType.subtract
                )
                nc.gpsimd.tensor_tensor(
                    ki[:], p_t["P3"][:], p_t["P4"][:], op=mybir.AluOpType.add
                )

                co = CW_OFF[gi_i]
                lhsT_nat = cw_s[:Kg, co : co + gi.n_nat]
                lhsT_rev = cw_s[:Kg, co + gi.n_nat : co + Mg]

                n_nat = gi.n_nat
                n_rev = Mg - n_nat
                for comp, ktile in (("r", kr), ("i", ki)):
                    sbo = outp.tile([Mg, 1024], dt.float32, tag="sbo")
                    for half in range(2):
                        # natural cols: free (t1,t2); this half: t1 in [16h,16h+16)
                        nat_rhs = ktile[:].rearrange("p (a b) -> p a b", a=32)[
                            :, half * 16 : (half + 1) * 16, :
                        ]
                        pso = psO.tile([n_nat, 512], dt.float32, tag="pso")
                        nc.tensor.matmul(
                            pso[:],
                            lhsT_nat,
                            nat_rhs,
                            start=True,
                            stop=True,
                        )
                        nc.scalar.copy(
                            sbo[:n_nat, half * 512 : (half + 1) * 512], pso[:]
                        )
                        if n_rev:
                            # reversed: value at (t1,t2) = tile[(t2,t1)]
                            rev_rhs = ktile[:].rearrange("p (a b) -> p b a", a=32)[
                                :, half * 16 : (half + 1) * 16, :
                            ]
                            psr = psO.tile([n_rev, 512], dt.float32, tag="psr")
                            nc.tensor.matmul(
                                psr[:],
                                lhsT_rev,
                                rev_rhs,
                                start=True,
                                stop=True,
                            )
                            nc.scalar.copy(
                                sbo[n_nat:, half * 512 : (half + 1) * 512], psr[:]
                            )
                    # 8 phase transposes: u = 8*p + r
                    pst = psT.tile([128, 1024], dt.float32, tag="pst")
                    for r in range(8):
                        src = sbo[:].rearrange("p (k r) -> p r k", r=8)[:, r, :]
                        nc.tensor.transpose(
                            pst[:, r * 128 : r * 128 + Mg],
                            src,
                            id_s[:Mg, :Mg],
                        )
                    # scatter copies into ST tiles
                    c = 0 if comp == "r" else 1
                    for bi, (col0, l1, l2, l, rev) in enumerate(gi.blocks):
                        Kl = 2 * l + 1
                        q = PAIRS_OF_L[l].index((l1, l2))
                        stt = st_tiles[l]
                        # dest [128, (r=8: step Kl*2), (m: step 2)] at q, c
                        dest = bass.AP(
                            stt[:].tensor,
                            stt[:].offset + q * 8 * Kl * 2 + c,
                            [
                                [stt[:].ap[0][0], 128],
                                [Kl * 2, 8],
                                [2, Kl],
                            ],
                        )
                        src = bass.AP(
                            pst[:].tensor,
                            pst[:].offset + col0,
                            [
                                [pst[:].ap[0][0], 128],
                                [128, 8],
                                [1, Kl],
                            ],
                        )
                        if bi % 2 == 0:
                            nc.vector.tensor_copy(dest, src)
                        else:
                            nc.scalar.copy(dest, src)
            # output DMAs
            for l in range(NL):
                Kl = 2 * l + 1
                npair = len(PAIRS_OF_L[l])
                stt = st_tiles[l]
                run = 8 * Kl * 2
                dest = bass.AP(
                    out_d.ap().tensor,
                    b * OUT_N * 2 + LBLOCK_BASE[l],
                    [
                        [run, 128],  # p: u-octet stride
                        [1024 * Kl * 2, npair],  # q
                        [1, run],
                    ],
                )
                src = bass.AP(
                    stt[:].tensor,
                    stt[:].offset,
                    [
                        [stt[:].ap[0][0], 128],
                        [run, npair],
                        [1, run],
                    ],
                )
                nc.sync.dma_start(dest, src)

    nc.compile()
    return nc


def _get_compiled():
    global _COMPILED
    if _COMPILED is None:
        _COMPILED = _build()
    return _COMPILED


_EXEC = None


def _get_exec():
    """Build the sharded PJRT executable once (mirrors bass2jax.run_bass_via_pjrt)."""
    global _EXEC
    if _EXEC is not None:
        return _EXEC
    import jax
    import jax.numpy as jnp
    from jax.sharding import Mesh, PartitionSpec
    from jax.experimental.shard_map import shard_map
    from concourse import bass2jax, mybir as _mybir

    bass2jax.install_neuronx_cc_hook()
    nc = _get_compiled()

    part_name = nc.partition_id_tensor.name if nc.partition_id_tensor else None
    in_names, out_names, out_avals = [], [], []
    for alloc in nc.m.functions[0].allocations:
        if not isinstance(alloc, _mybir.MemoryLocationSet):
            continue
        name = alloc.memorylocations[0].name
        if alloc.kind == "ExternalInput":
            if name != part_name:
                in_names.append(name)
        elif alloc.kind == "ExternalOutput":
            shape = tuple(alloc.tensor_shape)
            out_avals.append(jax.core.ShapedArray(shape, _mybir.dt.np(alloc.dtype)))
            out_names.append(name)
    n_params = len(in_names)
    n_outs = len(out_avals)
    all_in_names = in_names + out_names
    if part_name is not None:
        all_in_names.append(part_name)

    def _body(*args):
        operands = list(args)
        if part_name is not None:
            operands.append(bass2jax.partition_id_tensor())
        outs = bass2jax._bass_exec_p.bind(
            *operands,
            out_avals=tuple(out_avals),
            in_names=tuple(all_in_names),
            out_names=tuple(out_names),
            lowering_input_output_aliases=(),
            sim_require_finite=True,
            sim_require_nnan=True,
            nc=nc,
        )
        return tuple(outs)

    devices = jax.devices()[:N_CORES]
    mesh = Mesh(np.asarray(devices), ("core",))
    donate = tuple(range(n_params, n_params + n_outs))
    sharded = jax.jit(
        shard_map(
            _body,
            mesh=mesh,
            in_specs=(PartitionSpec("core"),) * (n_params + n_outs),
            out_specs=(PartitionSpec("core"),) * n_outs,
            check_rep=False,
        ),
        donate_argnums=donate,
        keep_unused=True,
    )
    # device-resident zero output buffers + on-device cloner (avoids a
    # 200MB host->device upload per call; donation consumes the clone)
    zeros_host = [
        np.zeros((N_CORES * a.shape[0], *a.shape[1:]), a.dtype) for a in out_avals
    ]
    zspec = jax.sharding.NamedSharding(mesh, PartitionSpec("core"))
    zeros_dev = [jax.device_put(z, zspec) for z in zeros_host]
    clone = jax.jit(lambda *zs: tuple(jnp.copy(z) for z in zs))

    _EXEC = (sharded, clone, zeros_dev, in_names, out_names, out_avals, zspec)
    return _EXEC


def _run_device(fs: np.ndarray):
    """Upload inputs, run the sharded executable; returns device arrays."""
    import jax

    sharded, clone, zeros_dev, in_names, out_names, out_avals, zspec = _get_exec()
    per_core = {
        "fs": fs.reshape(N_CORES, B_LOC, TOTAL_IN),
        "repw": np.broadcast_to(REP_W, (N_CORES, *REP_W.shape)),
        "cw": np.broadcast_to(CW, (N_CORES, *CW.shape)),
        "ident": np.broadcast_to(IDENT, (N_CORES, *IDENT.shape)),
    }
    ins = [
        np.ascontiguousarray(per_core[n].reshape(-1, *per_core[n].shape[2:]))
        for n in in_names
    ]
    zcopies = clone(*zeros_dev)
    outs = sharded(*ins, *zcopies)
    return outs, out_names, out_avals


def kernel(fs: np.ndarray) -> np.ndarray:
    fs = np.asarray(fs, dtype=np.float32)
    outs, out_names, out_avals = _run_device(fs)
    out = np.asarray(outs[out_names.index("out")])
    return out.reshape(BATCH, OUT_N, 2)


if __name__ == "__main__":
    x = np.random.default_rng(0).standard_normal((BATCH, TOTAL_IN), dtype=np.float32)
    y = kernel(fs=x)
    print("out", y.shape, y.dtype)
